# revision 11
# baseline (speedup 1.0000x reference)
"""Trainium2 Bass kernel for nn_EncoderStack (dense transformer encoder layer).

Strategy (8 NeuronCores, single NEFF launch):
  Attention is head-parallel: each core owns 2 of 16 heads over all 4096
  tokens. scores^T = k q^T per head, softmax over the query axis (free
  dim), denominator folded into v, o^T accumulated in PSUM with both
  heads packed into one [128, T] tile. Per batch, an AllToAll exchanges
  o^T blocks so each core ends up with all 1024 attention features for
  its 256-token slice of that batch. Wo + residual + norm + FFN + norm
  then run token-parallel. b2 is folded into the W2 matmul via a ones
  row; 1/sqrt(var) is computed as exp(-0.5*ln(var)) so every scalar-
  engine op stays in one activation table set (no ~2.7us set switches).

  Emission is software-pipelined: qkv of batch 1 is interleaved into the
  ACT-bound attention of batch 0, and Wo/norm/W1 work for batch 0's
  tokens is interleaved into the attention of batch 1, keeping TensorE
  dense (HAM stays un-throttled).

All matmuls run bf16 with fp32 PSUM accumulation; normalization
statistics stay fp32.
"""

import numpy as np

B, T, D = 2, 2048, 1024
H, DK, DV = 16, 64, 64
FF = 4096
N_CORES = 8
P = 128
TOK = B * T
TPB = T // N_CORES    # 256 tokens per core per batch
HPC = H // N_CORES    # 2 heads per core
KT = D // P           # 8
FT = FF // P          # 32
ST = T // P           # 16
TT = TPB // P         # 2 token-tiles per core per batch

_CACHE = {}


def _build():
    import concourse.bacc as bacc
    import concourse.mybir as mybir
    from concourse import tile

    f32 = mybir.dt.float32
    bf16 = mybir.dt.bfloat16
    AX = mybir.AxisListType
    AF = mybir.ActivationFunctionType
    ALU = mybir.AluOpType

    nc = bacc.Bacc("TRN2", target_bir_lowering=False, debug=False,
                   enable_asserts=True, num_devices=N_CORES)

    xt_d = nc.dram_tensor("xt", [KT, P, TOK], bf16, kind="ExternalInput")
    xres_d = nc.dram_tensor("xres", [2 * TPB, D], f32, kind="ExternalInput")
    wqkv_d = nc.dram_tensor("wqkv", [KT, P, 384], bf16, kind="ExternalInput")
    wo_d = nc.dram_tensor("wo", [KT, P, D], bf16, kind="ExternalInput")
    w1_d = nc.dram_tensor("w1", [FT, KT, P, P], bf16, kind="ExternalInput")
    b1_d = nc.dram_tensor("b1", [P, FT], f32, kind="ExternalInput")
    w2_d = nc.dram_tensor("w2", [FT + 1, P, D], bf16, kind="ExternalInput")
    out_d = nc.dram_tensor("out", [2 * TPB, D], f32, kind="ExternalOutput")

    xres_r = xres_d.ap().rearrange("(a p) d -> a p d", p=P)
    out_r = out_d.ap().rearrange("(a p) d -> a p d", p=P)

    def drain(g):
        for _ in g:
            pass

    def zip2(main, filler, ratio=1):
        while True:
            try:
                next(main)
            except StopIteration:
                drain(filler)
                return
            for _ in range(ratio):
                try:
                    next(filler)
                except StopIteration:
                    drain(main)
                    return

    with tile.TileContext(nc) as tc:
        with tc.tile_pool(name="wts", bufs=1) as wts, \
             tc.tile_pool(name="small", bufs=6) as small, \
             tc.tile_pool(name="o1", bufs=2) as o1p, \
             tc.tile_pool(name="p1", bufs=2) as p1, \
             tc.tile_pool(name="p2", bufs=2) as p2, \
             tc.tile_pool(name="ps", bufs=2, space="PSUM") as psp, \
             tc.tile_pool(name="dram", bufs=1, space="DRAM") as dram:

            def dma3(dst2d, src_ap, a):
                nc.sync.dma_start(
                    dst2d.rearrange("p (a m) -> p a m", a=a),
                    src_ap.rearrange("a p m -> p a m"))

            wqkv_sb = wts.tile([P, KT * 384], bf16)
            dma3(wqkv_sb[:], wqkv_d.ap(), KT)
            wo_sb = wts.tile([P, KT * D], bf16)
            dma3(wo_sb[:], wo_d.ap(), KT)
            b1_sb = wts.tile([P, FT], f32)
            nc.sync.dma_start(b1_sb[:], b1_d.ap())
            ones_sb = wts.tile([P, TPB], bf16)
            nc.vector.memset(ones_sb[:], 0.0)
            nc.vector.memset(ones_sb[0:1, :], 1.0)

            a2a_in = [[dram.tile([N_CORES, 64, TPB], bf16, tag=f"ain{b}{h}",
                               name=f"ain{b}{h}") for h in range(HPC)]
                      for b in range(B)]
            a2a_out = [[dram.tile([N_CORES, 64, TPB], bf16, tag=f"aout{b}{h}",
                                name=f"aout{b}{h}") for h in range(HPC)]
                       for b in range(B)]

            def emit_a2a(b, h):
                nc.gpsimd.collective_compute(
                    "AllToAll", ALU.bypass,
                    replica_groups=[list(range(N_CORES))],
                    ins=[a2a_in[b][h].opt()], outs=[a2a_out[b][h].opt()])

            q_sb = [None, None]
            k_sb = [None, None]
            v_sb = [None, None]
            oall_sb = [None, None]
            out1b_sb = [None, None]
            out1T_sb = [None, None]
            h1T_sb = [None, None]

            def gen_qkv(b):
                xt_b = p1.tile([P, KT * T], bf16, tag="xt", bufs=1, name="xt_b")
                for kt in range(KT):
                    nc.sync.dma_start(xt_b[:, kt * T:(kt + 1) * T],
                                      xt_d.ap()[kt, :, b * T:(b + 1) * T])
                yield
                q_sb[b] = p1.tile([P, T], bf16, tag="q", name="q_sb")
                k_sb[b] = p1.tile([P, T], bf16, tag="k", name="k_sb")
                v_sb[b] = p1.tile([P, T], bf16, tag="v", name="v_sb")
                for dst, wofs in ((q_sb[b], 0), (k_sb[b], P)):
                    for half in range(2):
                        pt = psp.tile([P, 1024], f32, tag="mm", name="pt")
                        for kt in range(KT):
                            for c in range(2):
                                ofs = half * 1024 + c * 512
                                nc.tensor.matmul(
                                    pt[:, c * 512:(c + 1) * 512],
                                    wqkv_sb[:, kt * 384 + wofs: kt * 384 + wofs + P],
                                    xt_b[:, kt * T + ofs: kt * T + ofs + 512],
                                    start=(kt == 0), stop=(kt == KT - 1))
                            yield
                        nc.vector.tensor_copy(
                            dst[:, half * 1024: half * 1024 + 1024], pt[:])
                for st in range(ST):
                    pv = psp.tile([P, P], f32, tag="mm", name="pv")
                    for kt in range(KT):
                        nc.tensor.matmul(
                            pv[:],
                            xt_b[:, kt * T + st * P: kt * T + (st + 1) * P],
                            wqkv_sb[:, kt * 384 + 256: kt * 384 + 384],
                            start=(kt == 0), stop=(kt == KT - 1))
                        if kt % 4 == 3:
                            yield
                    nc.vector.tensor_copy(v_sb[b][:, st * P:(st + 1) * P], pv[:])

            def gen_attn(b):
                for h in range(HPC):
                    hofs = 64 * h
                    o_ps = psp.tile([64, T], f32, tag="o", bufs=1, name="o_ps")
                    for st in range(ST):
                        at_tiles = [None, None]
                        zp = small.tile([P, 2], f32, tag="zp", name="zp")
                        for half in range(2):
                            sc = psp.tile([P, 1024], f32, tag="mm", name="sc")
                            for c in range(2):
                                ofs = half * 1024 + c * 512
                                nc.tensor.matmul(
                                    sc[:, c * 512:(c + 1) * 512],
                                    k_sb[b][hofs:hofs + 64, st * P:(st + 1) * P],
                                    q_sb[b][hofs:hofs + 64, ofs: ofs + 512],
                                    start=True, stop=True)
                            at = p1.tile([P, 1024], bf16, tag="at", bufs=5, name="at")
                            nc.scalar.activation(at[:], sc[:], AF.Exp, scale=0.125,
                                                 accum_out=zp[:, half:half + 1])
                            at_tiles[half] = at
                        yield
                        zs = small.tile([P, 1], f32, tag="zs", name="zs")
                        nc.vector.tensor_add(zs[:], zp[:, 0:1], zp[:, 1:2])
                        zi = small.tile([P, 1], f32, tag="zi", name="zi")
                        nc.vector.reciprocal(zi[:], zs[:])
                        vp = small.tile([P, 64], bf16, tag="vp", name="vp")
                        nc.vector.tensor_scalar_mul(
                            vp[:], v_sb[b][:, st * P + hofs: st * P + hofs + 64], zi[:])
                        for half in range(2):
                            for c in range(2):
                                ofs = half * 1024 + c * 512
                                nc.tensor.matmul(
                                    o_ps[:, ofs: ofs + 512],
                                    vp[:],
                                    at_tiles[half][:, c * 512:(c + 1) * 512],
                                    start=(st == 0), stop=(st == ST - 1))
                        yield
                    ot = p1.tile([64, T], bf16, tag="ot", bufs=2, name="ot")
                    nc.vector.tensor_copy(ot[:], o_ps[:])
                    for j in range(N_CORES):
                        nc.sync.dma_start(a2a_in[b][h][j],
                                          ot[:, j * TPB:(j + 1) * TPB])
                    emit_a2a(b, h)

            i32 = mybir.dt.int32
            magic1 = wts.tile([P, 1], i32)
            nc.vector.memset(magic1[:], 0x5f3759e0)

            def norm_rows(y_ap, ssum, out_ap):
                negmean = small.tile([P, 1], f32, tag="st2", name="negmean")
                nc.scalar.mul(negmean[:], ssum[:], -1.0 / D)
                sq = p2.tile([P, D], bf16, tag="sq", bufs=1, name="sq")
                ssq = small.tile([P, 1], f32, tag="st4", name="ssq")
                nc.scalar.activation(sq[:], y_ap, AF.Square,
                                     bias=negmean[:], accum_out=ssq[:])
                # istd = rsqrt(ssq/(D-1)) via magic-constant seed + 2 Newton
                # steps, all tiny [P,1] DVE ops (keeps ScalarE in one
                # activation table set - no ACT_TABLE_LOAD churn).
                v = small.tile([P, 1], f32, tag="st5", name="v")
                nc.vector.tensor_scalar_mul(v[:], ssq[:], 1.0 / (D - 1))
                yh = small.tile([P, 1], i32, tag="st6", name="yh")
                nc.vector.tensor_scalar(yh[:], v[:].bitcast(i32), 1, None,
                                        op0=ALU.logical_shift_right)
                yn = small.tile([P, 1], i32, tag="st12", name="yn")
                nc.vector.tensor_scalar(yn[:], yh[:], -1, None,
                                        op0=ALU.bitwise_xor)
                y0 = small.tile([P, 1], i32, tag="st7", name="y0")
                nc.vector.tensor_add(y0[:], yn[:], magic1[:])
                istd = y0[:].bitcast(f32)
                for _ in range(2):
                    aa = small.tile([P, 1], f32, tag="st8", name="aa")
                    nc.vector.tensor_mul(aa[:], istd, istd)
                    bb = small.tile([P, 1], f32, tag="st9", name="bb")
                    nc.vector.tensor_mul(bb[:], v[:], aa[:])
                    cc = small.tile([P, 1], f32, tag="st10", name="cc")
                    nc.vector.tensor_scalar(cc[:], bb[:], -0.5, 1.5,
                                            op0=ALU.mult, op1=ALU.add)
                    ny = small.tile([P, 1], f32, tag="st11", name="ny")
                    nc.vector.tensor_mul(ny[:], cc[:], istd)
                    istd = ny[:]
                nc.vector.tensor_scalar(out_ap, y_ap, negmean[:], istd,
                                        op0=ALU.add, op1=ALU.mult)

            def gen_p2a(b):
                # gather attention features for my tokens of batch b
                oall_sb[b] = o1p.tile([P, KT * TPB], bf16, tag="oall", name="oall_sb")
                for kt in range(KT):
                    for h in range(HPC):
                        nc.sync.dma_start(
                            oall_sb[b][64 * h:64 * h + 64, kt * TPB:(kt + 1) * TPB],
                            a2a_out[b][h][kt])
                out1b_sb[b] = o1p.tile([P, TT * D], bf16, tag="out1b", name="out1b_sb")
                out1T_sb[b] = o1p.tile([P, KT * TPB], bf16, tag="out1t", name="out1T_sb")
                for tt in range(TT):
                    pw = psp.tile([P, D], f32, tag="mm", name="pw")
                    for kt in range(KT):
                        for c in range(2):
                            nc.tensor.matmul(
                                pw[:, c * 512:(c + 1) * 512],
                                oall_sb[b][:, kt * TPB + tt * P: kt * TPB + (tt + 1) * P],
                                wo_sb[:, kt * D + c * 512: kt * D + (c + 1) * 512],
                                start=(kt == 0), stop=(kt == KT - 1))
                        if kt % 2 == 1:
                            yield
                    xr = p2.tile([P, D], f32, tag="xr", name="xr")
                    nc.sync.dma_start(xr[:], xres_r[b * TT + tt])
                    y = p2.tile([P, D], f32, tag="y", name="y")
                    ssum = small.tile([P, 1], f32, tag="st1", name="ssum")
                    nc.vector.scalar_tensor_tensor(
                        y[:], pw[:], 0.0, xr[:], op0=ALU.add, op1=ALU.add,
                        accum_out=ssum[:])
                    norm_rows(y[:], ssum, out1b_sb[b][:, tt * D:(tt + 1) * D])
                    yield
                    for kt in range(KT):
                        nc.sync.dma_start_transpose(
                            out1T_sb[b][:, kt * TPB + tt * P: kt * TPB + (tt + 1) * P],
                            out1b_sb[b][:, tt * D + kt * P: tt * D + (kt + 1) * P])
                    yield
                h1T_sb[b] = p2.tile([P, FT * TPB], bf16, tag="h1t", bufs=1, name="h1T_sb")
                for ft in range(FT):
                    w1s = p2.tile([P, KT * P], bf16, tag="w1s", name="w1s")
                    dma3(w1s[:], w1_d.ap()[ft], KT)
                    ph = psp.tile([P, TPB], f32, tag="mm", name="ph")
                    for kt in range(KT):
                        nc.tensor.matmul(
                            ph[:],
                            w1s[:, kt * P:(kt + 1) * P],
                            out1T_sb[b][:, kt * TPB:(kt + 1) * TPB],
                            start=(kt == 0), stop=(kt == KT - 1))
                        if kt % 4 == 3:
                            yield
                    # relu(x + b1) on DVE: (ph + b1) max 0, bf16 out
                    nc.vector.tensor_scalar(
                        h1T_sb[b][:, ft * TPB:(ft + 1) * TPB], ph[:],
                        b1_sb[:, ft:ft + 1], 0.0, op0=ALU.add, op1=ALU.max)

            def gen_ffn(b):
                pfs = [psp.tile([P, D], f32, tag="mm", name=f"pf{tt}")
                       for tt in range(TT)]
                for ft in range(FT + 1):
                    w2s = p2.tile([P, D], bf16, tag="w2s", bufs=4, name="w2s")
                    nc.sync.dma_start(w2s[:], w2_d.ap()[ft])
                    last = ft == FT
                    for tt in range(TT):
                        lhsT = (ones_sb[:, tt * P:(tt + 1) * P] if last else
                                h1T_sb[b][:, ft * TPB + tt * P: ft * TPB + (tt + 1) * P])
                        for c in range(2):
                            nc.tensor.matmul(
                                pfs[tt][:, c * 512:(c + 1) * 512],
                                lhsT,
                                w2s[:, c * 512:(c + 1) * 512],
                                start=(ft == 0), stop=last)
                    if ft % 2 == 1:
                        yield
                for tt in range(TT):
                    y2 = p2.tile([P, D], f32, tag="y", name="y2")
                    ssum = small.tile([P, 1], f32, tag="st1", name="ssum2")
                    nc.vector.scalar_tensor_tensor(
                        y2[:], pfs[tt][:], 0.0, out1b_sb[b][:, tt * D:(tt + 1) * D],
                        op0=ALU.add, op1=ALU.add, accum_out=ssum[:])
                    o2 = p2.tile([P, D], f32, tag="o2", name="o2")
                    norm_rows(y2[:], ssum, o2[:])
                    nc.sync.dma_start(out_r[b * TT + tt], o2[:])
                    yield

            # ---------------- emission schedule ----------------
            drain(gen_qkv(0))
            attn0 = gen_attn(0)
            qkv1 = gen_qkv(1)
            next(qkv1)          # emit xt(b1) load early
            for _ in range(12):  # ~3 s-tiles solo while xt(b1) streams in
                next(attn0)
            zip2(attn0, qkv1, ratio=2)
            attn1 = gen_attn(1)
            for _ in range(32):   # full h0 solo: let both A2A-0 halves land
                next(attn1)
            zip2(attn1, gen_p2a(0), ratio=2)
            drain(gen_ffn(0))
            drain(gen_p2a(1))
            drain(gen_ffn(1))

    nc.compile()
    return nc


def _get_nc():
    if "nc" not in _CACHE:
        _CACHE["nc"] = _build()
    return _CACHE["nc"]


def _prep_inputs(x, Wq, Wk, Wv, Wo, W1, b1, W2, b2):
    import ml_dtypes
    bf = ml_dtypes.bfloat16
    x = np.asarray(x, np.float32)
    x2 = np.ascontiguousarray(x.reshape(TOK, D))
    xt = np.ascontiguousarray(x2.T).astype(bf).reshape(KT, P, TOK)
    wo8 = np.ascontiguousarray(np.asarray(Wo, np.float32).astype(bf).reshape(KT, P, D))
    w1t = np.ascontiguousarray(
        np.asarray(W1, np.float32).astype(bf).reshape(KT, P, FT, P).transpose(2, 0, 1, 3))
    b2blk = np.zeros((1, P, D), np.float32)
    b2blk[0, 0, :] = np.asarray(b2, np.float32)
    w2t = np.ascontiguousarray(np.concatenate(
        [np.asarray(W2, np.float32).reshape(FT, P, D), b2blk], axis=0).astype(bf))
    b1t = np.ascontiguousarray(np.asarray(b1, np.float32).reshape(FT, P).T)
    Wq = np.asarray(Wq, np.float32)
    Wk = np.asarray(Wk, np.float32)
    Wv = np.asarray(Wv, np.float32)
    in_maps = []
    for c in range(N_CORES):
        h0 = HPC * c
        wqkv = np.concatenate(
            [Wq[h0], Wq[h0 + 1], Wk[h0], Wk[h0 + 1], Wv[h0], Wv[h0 + 1]],
            axis=1).astype(bf)
        wqkv = np.ascontiguousarray(wqkv.reshape(KT, P, 384))
        xres = np.ascontiguousarray(np.concatenate(
            [x2[c * TPB:(c + 1) * TPB],
             x2[T + c * TPB: T + (c + 1) * TPB]], axis=0))
        in_maps.append({
            "xt": xt, "xres": xres, "wqkv": wqkv, "wo": wo8,
            "w1": w1t, "b1": b1t, "w2": w2t,
        })
    return in_maps


def _assemble(results):
    out = np.empty((TOK, D), np.float32)
    for c in range(N_CORES):
        r = np.asarray(results[c]["out"], np.float32)
        out[c * TPB:(c + 1) * TPB] = r[:TPB]
        out[T + c * TPB: T + (c + 1) * TPB] = r[TPB:]
    return out.reshape(B, T, D)


def kernel(x, Wq, Wk, Wv, Wo, W1, b1, W2, b2):
    from concourse.bass_utils import run_bass_kernel_spmd
    nc = _get_nc()
    in_maps = _prep_inputs(x, Wq, Wk, Wv, Wo, W1, b1, W2, b2)
    res = run_bass_kernel_spmd(nc, in_maps, core_ids=list(range(N_CORES)))
    return _assemble(res.results)


# revision 12
# speedup vs baseline: 1.0470x; 1.0470x over previous
"""Trainium2 Bass kernel for nn_EncoderStack (dense transformer encoder layer).

Strategy (8 NeuronCores, single NEFF launch):
  Attention is head-parallel: each core owns 2 of 16 heads over all 4096
  tokens. scores^T = k q^T per head, softmax over the query axis (free
  dim), denominator folded into v, o^T accumulated in PSUM with both
  heads packed into one [128, T] tile. Per batch, an AllToAll exchanges
  o^T blocks so each core ends up with all 1024 attention features for
  its 256-token slice of that batch. Wo + residual + norm + FFN + norm
  then run token-parallel. b2 is folded into the W2 matmul via a ones
  row; 1/sqrt(var) is computed as exp(-0.5*ln(var)) so every scalar-
  engine op stays in one activation table set (no ~2.7us set switches).

  Emission is software-pipelined: qkv of batch 1 is interleaved into the
  ACT-bound attention of batch 0, and Wo/norm/W1 work for batch 0's
  tokens is interleaved into the attention of batch 1, keeping TensorE
  dense (HAM stays un-throttled).

All matmuls run bf16 with fp32 PSUM accumulation; normalization
statistics stay fp32.
"""

import numpy as np

B, T, D = 2, 2048, 1024
H, DK, DV = 16, 64, 64
FF = 4096
N_CORES = 8
P = 128
TOK = B * T
TPB = T // N_CORES    # 256 tokens per core per batch
HPC = H // N_CORES    # 2 heads per core
KT = D // P           # 8
FT = FF // P          # 32
ST = T // P           # 16
TT = TPB // P         # 2 token-tiles per core per batch

_CACHE = {}


def _build():
    import concourse.bacc as bacc
    import concourse.mybir as mybir
    from concourse import tile

    f32 = mybir.dt.float32
    bf16 = mybir.dt.bfloat16
    AX = mybir.AxisListType
    AF = mybir.ActivationFunctionType
    ALU = mybir.AluOpType

    nc = bacc.Bacc("TRN2", target_bir_lowering=False, debug=False,
                   enable_asserts=True, num_devices=N_CORES)

    xt_d = nc.dram_tensor("xt", [KT, P, TOK], bf16, kind="ExternalInput")
    xres_d = nc.dram_tensor("xres", [2 * TPB, D], f32, kind="ExternalInput")
    wqkv_d = nc.dram_tensor("wqkv", [KT, P, 384], bf16, kind="ExternalInput")
    wo_d = nc.dram_tensor("wo", [KT, P, D], bf16, kind="ExternalInput")
    w1_d = nc.dram_tensor("w1", [FT, KT, P, P], bf16, kind="ExternalInput")
    b1_d = nc.dram_tensor("b1", [P, FT], f32, kind="ExternalInput")
    w2_d = nc.dram_tensor("w2", [FT + 1, P, D], bf16, kind="ExternalInput")
    out_d = nc.dram_tensor("out", [2 * TPB, D], f32, kind="ExternalOutput")

    xres_r = xres_d.ap().rearrange("(a p) d -> a p d", p=P)
    out_r = out_d.ap().rearrange("(a p) d -> a p d", p=P)

    def drain(g):
        for _ in g:
            pass

    def zip2(main, filler, ratio=1):
        while True:
            try:
                next(main)
            except StopIteration:
                drain(filler)
                return
            for _ in range(ratio):
                try:
                    next(filler)
                except StopIteration:
                    drain(main)
                    return

    with tile.TileContext(nc) as tc:
        with tc.tile_pool(name="wts", bufs=1) as wts, \
             tc.tile_pool(name="small", bufs=6) as small, \
             tc.tile_pool(name="o1", bufs=2) as o1p, \
             tc.tile_pool(name="p1", bufs=2) as p1, \
             tc.tile_pool(name="p2", bufs=2) as p2, \
             tc.tile_pool(name="ps", bufs=2, space="PSUM") as psp, \
             tc.tile_pool(name="dram", bufs=1, space="DRAM") as dram:

            def dma3(dst2d, src_ap, a):
                nc.sync.dma_start(
                    dst2d.rearrange("p (a m) -> p a m", a=a),
                    src_ap.rearrange("a p m -> p a m"))

            wqkv_sb = wts.tile([P, KT * 384], bf16)
            dma3(wqkv_sb[:], wqkv_d.ap(), KT)
            wo_sb = wts.tile([P, KT * D], bf16)
            dma3(wo_sb[:], wo_d.ap(), KT)
            b1_sb = wts.tile([P, FT], f32)
            nc.sync.dma_start(b1_sb[:], b1_d.ap())
            ones_sb = wts.tile([P, TPB], bf16)
            nc.vector.memset(ones_sb[:], 0.0)
            nc.vector.memset(ones_sb[0:1, :], 1.0)

            a2a_in = [[dram.tile([N_CORES, 64, TPB], bf16, tag=f"ain{b}{h}",
                               name=f"ain{b}{h}") for h in range(HPC)]
                      for b in range(B)]
            a2a_out = [[dram.tile([N_CORES, 64, TPB], bf16, tag=f"aout{b}{h}",
                                name=f"aout{b}{h}") for h in range(HPC)]
                       for b in range(B)]

            def emit_a2a(b, h):
                nc.gpsimd.collective_compute(
                    "AllToAll", ALU.bypass,
                    replica_groups=[list(range(N_CORES))],
                    ins=[a2a_in[b][h].opt()], outs=[a2a_out[b][h].opt()])

            q_sb = [None, None]
            k_sb = [None, None]
            v_sb = [None, None]
            oall_sb = [None, None]
            out1b_sb = [None, None]
            out1T_sb = [None, None]
            h1T_sb = [None, None]

            def gen_qkv(b):
                xt_b = p1.tile([P, KT * T], bf16, tag="xt", bufs=1, name="xt_b")
                for kt in range(KT):
                    nc.sync.dma_start(xt_b[:, kt * T:(kt + 1) * T],
                                      xt_d.ap()[kt, :, b * T:(b + 1) * T])
                yield
                q_sb[b] = p1.tile([P, T], bf16, tag="q", name="q_sb")
                k_sb[b] = p1.tile([P, T], bf16, tag="k", name="k_sb")
                v_sb[b] = p1.tile([P, T], bf16, tag="v", name="v_sb")
                for dst, wofs in ((q_sb[b], 0), (k_sb[b], P)):
                    for half in range(2):
                        pt = psp.tile([P, 1024], f32, tag="mm", name="pt")
                        for kt in range(KT):
                            for c in range(2):
                                ofs = half * 1024 + c * 512
                                nc.tensor.matmul(
                                    pt[:, c * 512:(c + 1) * 512],
                                    wqkv_sb[:, kt * 384 + wofs: kt * 384 + wofs + P],
                                    xt_b[:, kt * T + ofs: kt * T + ofs + 512],
                                    start=(kt == 0), stop=(kt == KT - 1))
                            yield
                        nc.vector.tensor_copy(
                            dst[:, half * 1024: half * 1024 + 1024], pt[:])
                for st in range(ST):
                    pv = psp.tile([P, P], f32, tag="mm", name="pv")
                    for kt in range(KT):
                        nc.tensor.matmul(
                            pv[:],
                            xt_b[:, kt * T + st * P: kt * T + (st + 1) * P],
                            wqkv_sb[:, kt * 384 + 256: kt * 384 + 384],
                            start=(kt == 0), stop=(kt == KT - 1))
                        if kt % 4 == 3:
                            yield
                    nc.vector.tensor_copy(v_sb[b][:, st * P:(st + 1) * P], pv[:])

            def gen_attn(b):
                for h in range(HPC):
                    hofs = 64 * h
                    o_ps = psp.tile([64, T], f32, tag="o", bufs=1, name="o_ps")
                    for st in range(ST):
                        at_tiles = [None, None]
                        zp = small.tile([P, 2], f32, tag="zp", name="zp")
                        for half in range(2):
                            sc = psp.tile([P, 1024], f32, tag="mm", name="sc")
                            for c in range(2):
                                ofs = half * 1024 + c * 512
                                nc.tensor.matmul(
                                    sc[:, c * 512:(c + 1) * 512],
                                    k_sb[b][hofs:hofs + 64, st * P:(st + 1) * P],
                                    q_sb[b][hofs:hofs + 64, ofs: ofs + 512],
                                    start=True, stop=True)
                            at = p1.tile([P, 1024], bf16, tag="at", bufs=5, name="at")
                            nc.scalar.activation(at[:], sc[:], AF.Exp, scale=0.125,
                                                 accum_out=zp[:, half:half + 1])
                            at_tiles[half] = at
                        yield
                        zs = small.tile([P, 1], f32, tag="zs", name="zs")
                        nc.vector.tensor_add(zs[:], zp[:, 0:1], zp[:, 1:2])
                        zi = small.tile([P, 1], f32, tag="zi", name="zi")
                        nc.vector.reciprocal(zi[:], zs[:])
                        vp = small.tile([P, 64], bf16, tag="vp", name="vp")
                        nc.vector.tensor_scalar_mul(
                            vp[:], v_sb[b][:, st * P + hofs: st * P + hofs + 64], zi[:])
                        for half in range(2):
                            for c in range(2):
                                ofs = half * 1024 + c * 512
                                nc.tensor.matmul(
                                    o_ps[:, ofs: ofs + 512],
                                    vp[:],
                                    at_tiles[half][:, c * 512:(c + 1) * 512],
                                    start=(st == 0), stop=(st == ST - 1))
                        yield
                    ot = p1.tile([64, T], bf16, tag="ot", bufs=2, name="ot")
                    nc.vector.tensor_copy(ot[:], o_ps[:])
                    for j in range(N_CORES):
                        nc.sync.dma_start(a2a_in[b][h][j],
                                          ot[:, j * TPB:(j + 1) * TPB])
                    emit_a2a(b, h)

            i32 = mybir.dt.int32
            magic1 = wts.tile([P, 1], i32)
            nc.vector.memset(magic1[:], 0x5f3759e0)

            def norm_rows(y_ap, ssum, out_ap):
                negmean = small.tile([P, 1], f32, tag="st2", name="negmean")
                nc.scalar.mul(negmean[:], ssum[:], -1.0 / D)
                sq = p2.tile([P, D], bf16, tag="sq", bufs=1, name="sq")
                ssq = small.tile([P, 1], f32, tag="st4", name="ssq")
                nc.scalar.activation(sq[:], y_ap, AF.Square,
                                     bias=negmean[:], accum_out=ssq[:])
                # istd = rsqrt(ssq/(D-1)) via magic-constant seed + 2 Newton
                # steps, all tiny [P,1] DVE ops (keeps ScalarE in one
                # activation table set - no ACT_TABLE_LOAD churn).
                v = small.tile([P, 1], f32, tag="st5", name="v")
                nc.vector.tensor_scalar_mul(v[:], ssq[:], 1.0 / (D - 1))
                yh = small.tile([P, 1], i32, tag="st6", name="yh")
                nc.vector.tensor_scalar(yh[:], v[:].bitcast(i32), 1, None,
                                        op0=ALU.logical_shift_right)
                yn = small.tile([P, 1], i32, tag="st12", name="yn")
                nc.vector.tensor_scalar(yn[:], yh[:], -1, None,
                                        op0=ALU.bitwise_xor)
                y0 = small.tile([P, 1], i32, tag="st7", name="y0")
                nc.vector.tensor_add(y0[:], yn[:], magic1[:])
                istd = y0[:].bitcast(f32)
                for _ in range(2):
                    aa = small.tile([P, 1], f32, tag="st8", name="aa")
                    nc.vector.tensor_mul(aa[:], istd, istd)
                    bb = small.tile([P, 1], f32, tag="st9", name="bb")
                    nc.vector.tensor_mul(bb[:], v[:], aa[:])
                    cc = small.tile([P, 1], f32, tag="st10", name="cc")
                    nc.vector.tensor_scalar(cc[:], bb[:], -0.5, 1.5,
                                            op0=ALU.mult, op1=ALU.add)
                    ny = small.tile([P, 1], f32, tag="st11", name="ny")
                    nc.vector.tensor_mul(ny[:], cc[:], istd)
                    istd = ny[:]
                nc.vector.tensor_scalar(out_ap, y_ap, negmean[:], istd,
                                        op0=ALU.add, op1=ALU.mult)

            def gen_p2a(b):
                # gather attention features for my tokens of batch b
                oall_sb[b] = o1p.tile([P, KT * TPB], bf16, tag="oall", name="oall_sb")
                for kt in range(KT):
                    for h in range(HPC):
                        nc.sync.dma_start(
                            oall_sb[b][64 * h:64 * h + 64, kt * TPB:(kt + 1) * TPB],
                            a2a_out[b][h][kt])
                out1b_sb[b] = o1p.tile([P, TT * D], bf16, tag="out1b", name="out1b_sb")
                out1T_sb[b] = o1p.tile([P, KT * TPB], bf16, tag="out1t", name="out1T_sb")
                for tt in range(TT):
                    pw = psp.tile([P, D], f32, tag="mm", name="pw")
                    for kt in range(KT):
                        for c in range(2):
                            nc.tensor.matmul(
                                pw[:, c * 512:(c + 1) * 512],
                                oall_sb[b][:, kt * TPB + tt * P: kt * TPB + (tt + 1) * P],
                                wo_sb[:, kt * D + c * 512: kt * D + (c + 1) * 512],
                                start=(kt == 0), stop=(kt == KT - 1))
                        if kt % 2 == 1:
                            yield
                    xr = p2.tile([P, D], f32, tag="xr", name="xr")
                    nc.sync.dma_start(xr[:], xres_r[b * TT + tt])
                    y = p2.tile([P, D], f32, tag="y", name="y")
                    ssum = small.tile([P, 1], f32, tag="st1", name="ssum")
                    nc.vector.scalar_tensor_tensor(
                        y[:], pw[:], 0.0, xr[:], op0=ALU.add, op1=ALU.add,
                        accum_out=ssum[:])
                    norm_rows(y[:], ssum, out1b_sb[b][:, tt * D:(tt + 1) * D])
                    yield
                    for kt in range(KT):
                        nc.sync.dma_start_transpose(
                            out1T_sb[b][:, kt * TPB + tt * P: kt * TPB + (tt + 1) * P],
                            out1b_sb[b][:, tt * D + kt * P: tt * D + (kt + 1) * P])
                    yield
                h1T_sb[b] = p2.tile([P, FT * TPB], bf16, tag="h1t", bufs=1, name="h1T_sb")
                for ft in range(FT):
                    w1s = p2.tile([P, KT * P], bf16, tag="w1s", name="w1s")
                    dma3(w1s[:], w1_d.ap()[ft], KT)
                    ph = psp.tile([P, TPB], f32, tag="mm", name="ph")
                    for kt in range(KT):
                        nc.tensor.matmul(
                            ph[:],
                            w1s[:, kt * P:(kt + 1) * P],
                            out1T_sb[b][:, kt * TPB:(kt + 1) * TPB],
                            start=(kt == 0), stop=(kt == KT - 1))
                        if kt % 4 == 3:
                            yield
                    # relu(x + b1) on DVE: (ph + b1) max 0, bf16 out
                    nc.vector.tensor_scalar(
                        h1T_sb[b][:, ft * TPB:(ft + 1) * TPB], ph[:],
                        b1_sb[:, ft:ft + 1], 0.0, op0=ALU.add, op1=ALU.max)

            def gen_ffn(b):
                pfs = [psp.tile([P, D], f32, tag="mm", name=f"pf{tt}")
                       for tt in range(TT)]
                for ft in range(FT + 1):
                    w2s = p2.tile([P, D], bf16, tag="w2s", bufs=4, name="w2s")
                    nc.sync.dma_start(w2s[:], w2_d.ap()[ft])
                    last = ft == FT
                    for tt in range(TT):
                        lhsT = (ones_sb[:, tt * P:(tt + 1) * P] if last else
                                h1T_sb[b][:, ft * TPB + tt * P: ft * TPB + (tt + 1) * P])
                        for c in range(2):
                            nc.tensor.matmul(
                                pfs[tt][:, c * 512:(c + 1) * 512],
                                lhsT,
                                w2s[:, c * 512:(c + 1) * 512],
                                start=(ft == 0), stop=last)
                    if ft % 2 == 1:
                        yield
                for tt in range(TT):
                    y2 = p2.tile([P, D], f32, tag="y", name="y2")
                    ssum = small.tile([P, 1], f32, tag="st1", name="ssum2")
                    nc.vector.scalar_tensor_tensor(
                        y2[:], pfs[tt][:], 0.0, out1b_sb[b][:, tt * D:(tt + 1) * D],
                        op0=ALU.add, op1=ALU.add, accum_out=ssum[:])
                    o2 = p2.tile([P, D], f32, tag="o2", name="o2")
                    norm_rows(y2[:], ssum, o2[:])
                    nc.sync.dma_start(out_r[b * TT + tt], o2[:])
                    yield

            # ---------------- emission schedule ----------------
            drain(gen_qkv(0))
            attn0 = gen_attn(0)
            qkv1 = gen_qkv(1)
            next(qkv1)          # emit xt(b1) load early
            for _ in range(12):  # ~3 s-tiles solo while xt(b1) streams in
                next(attn0)
            zip2(attn0, qkv1, ratio=2)
            attn1 = gen_attn(1)
            for _ in range(18):   # solo prefix: let the A2A-0 halves land
                next(attn1)
            zip2(attn1, gen_p2a(0), ratio=3)
            drain(gen_ffn(0))
            drain(gen_p2a(1))
            drain(gen_ffn(1))

    nc.compile()
    return nc


def _get_nc():
    if "nc" not in _CACHE:
        _CACHE["nc"] = _build()
    return _CACHE["nc"]


def _prep_inputs(x, Wq, Wk, Wv, Wo, W1, b1, W2, b2):
    import ml_dtypes
    bf = ml_dtypes.bfloat16
    x = np.asarray(x, np.float32)
    x2 = np.ascontiguousarray(x.reshape(TOK, D))
    xt = np.ascontiguousarray(x2.T).astype(bf).reshape(KT, P, TOK)
    wo8 = np.ascontiguousarray(np.asarray(Wo, np.float32).astype(bf).reshape(KT, P, D))
    w1t = np.ascontiguousarray(
        np.asarray(W1, np.float32).astype(bf).reshape(KT, P, FT, P).transpose(2, 0, 1, 3))
    b2blk = np.zeros((1, P, D), np.float32)
    b2blk[0, 0, :] = np.asarray(b2, np.float32)
    w2t = np.ascontiguousarray(np.concatenate(
        [np.asarray(W2, np.float32).reshape(FT, P, D), b2blk], axis=0).astype(bf))
    b1t = np.ascontiguousarray(np.asarray(b1, np.float32).reshape(FT, P).T)
    Wq = np.asarray(Wq, np.float32)
    Wk = np.asarray(Wk, np.float32)
    Wv = np.asarray(Wv, np.float32)
    in_maps = []
    for c in range(N_CORES):
        h0 = HPC * c
        wqkv = np.concatenate(
            [Wq[h0], Wq[h0 + 1], Wk[h0], Wk[h0 + 1], Wv[h0], Wv[h0 + 1]],
            axis=1).astype(bf)
        wqkv = np.ascontiguousarray(wqkv.reshape(KT, P, 384))
        xres = np.ascontiguousarray(np.concatenate(
            [x2[c * TPB:(c + 1) * TPB],
             x2[T + c * TPB: T + (c + 1) * TPB]], axis=0))
        in_maps.append({
            "xt": xt, "xres": xres, "wqkv": wqkv, "wo": wo8,
            "w1": w1t, "b1": b1t, "w2": w2t,
        })
    return in_maps


def _assemble(results):
    out = np.empty((TOK, D), np.float32)
    for c in range(N_CORES):
        r = np.asarray(results[c]["out"], np.float32)
        out[c * TPB:(c + 1) * TPB] = r[:TPB]
        out[T + c * TPB: T + (c + 1) * TPB] = r[TPB:]
    return out.reshape(B, T, D)


def kernel(x, Wq, Wk, Wv, Wo, W1, b1, W2, b2):
    from concourse.bass_utils import run_bass_kernel_spmd
    nc = _get_nc()
    in_maps = _prep_inputs(x, Wq, Wk, Wv, Wo, W1, b1, W2, b2)
    res = run_bass_kernel_spmd(nc, in_maps, core_ids=list(range(N_CORES)))
    return _assemble(res.results)


# revision 13
# speedup vs baseline: 1.0473x; 1.0003x over previous
"""Trainium2 Bass kernel for nn_EncoderStack (dense transformer encoder layer).

Strategy (8 NeuronCores, single NEFF launch):
  Attention is head-parallel: each core owns 2 of 16 heads over all 4096
  tokens. scores^T = k q^T per head, softmax over the query axis (free
  dim), denominator folded into v, o^T accumulated in PSUM with both
  heads packed into one [128, T] tile. Per batch, an AllToAll exchanges
  o^T blocks so each core ends up with all 1024 attention features for
  its 256-token slice of that batch. Wo + residual + norm + FFN + norm
  then run token-parallel. b2 is folded into the W2 matmul via a ones
  row; 1/sqrt(var) is computed as exp(-0.5*ln(var)) so every scalar-
  engine op stays in one activation table set (no ~2.7us set switches).

  Emission is software-pipelined: qkv of batch 1 is interleaved into the
  ACT-bound attention of batch 0, and Wo/norm/W1 work for batch 0's
  tokens is interleaved into the attention of batch 1, keeping TensorE
  dense (HAM stays un-throttled).

All matmuls run bf16 with fp32 PSUM accumulation; normalization
statistics stay fp32.
"""

import numpy as np

B, T, D = 2, 2048, 1024
H, DK, DV = 16, 64, 64
FF = 4096
N_CORES = 8
P = 128
TOK = B * T
TPB = T // N_CORES    # 256 tokens per core per batch
HPC = H // N_CORES    # 2 heads per core
KT = D // P           # 8
FT = FF // P          # 32
ST = T // P           # 16
TT = TPB // P         # 2 token-tiles per core per batch

_CACHE = {}


def _build():
    import concourse.bacc as bacc
    import concourse.mybir as mybir
    from concourse import tile

    f32 = mybir.dt.float32
    bf16 = mybir.dt.bfloat16
    AX = mybir.AxisListType
    AF = mybir.ActivationFunctionType
    ALU = mybir.AluOpType

    nc = bacc.Bacc("TRN2", target_bir_lowering=False, debug=False,
                   enable_asserts=True, num_devices=N_CORES)

    xt_d = nc.dram_tensor("xt", [KT, P, TOK], bf16, kind="ExternalInput")
    xres_d = nc.dram_tensor("xres", [2 * TPB, D], f32, kind="ExternalInput")
    wqkv_d = nc.dram_tensor("wqkv", [KT, P, 384], bf16, kind="ExternalInput")
    wo_d = nc.dram_tensor("wo", [KT, P, D], bf16, kind="ExternalInput")
    w1_d = nc.dram_tensor("w1", [FT, KT, P, P], bf16, kind="ExternalInput")
    b1_d = nc.dram_tensor("b1", [P, FT], f32, kind="ExternalInput")
    w2_d = nc.dram_tensor("w2", [FT + 1, P, D], bf16, kind="ExternalInput")
    out_d = nc.dram_tensor("out", [2 * TPB, D], f32, kind="ExternalOutput")

    xres_r = xres_d.ap().rearrange("(a p) d -> a p d", p=P)
    out_r = out_d.ap().rearrange("(a p) d -> a p d", p=P)

    def drain(g):
        for _ in g:
            pass

    def zip2(main, filler, ratio=1):
        while True:
            try:
                next(main)
            except StopIteration:
                drain(filler)
                return
            for _ in range(ratio):
                try:
                    next(filler)
                except StopIteration:
                    drain(main)
                    return

    with tile.TileContext(nc) as tc:
        with tc.tile_pool(name="wts", bufs=1) as wts, \
             tc.tile_pool(name="small", bufs=6) as small, \
             tc.tile_pool(name="o1", bufs=2) as o1p, \
             tc.tile_pool(name="p1", bufs=2) as p1, \
             tc.tile_pool(name="p2", bufs=2) as p2, \
             tc.tile_pool(name="ps", bufs=2, space="PSUM") as psp, \
             tc.tile_pool(name="dram", bufs=1, space="DRAM") as dram:

            def dma3(dst2d, src_ap, a):
                nc.sync.dma_start(
                    dst2d.rearrange("p (a m) -> p a m", a=a),
                    src_ap.rearrange("a p m -> p a m"))

            wqkv_sb = wts.tile([P, KT * 384], bf16)
            dma3(wqkv_sb[:], wqkv_d.ap(), KT)
            wo_sb = wts.tile([P, KT * D], bf16)
            dma3(wo_sb[:], wo_d.ap(), KT)
            b1_sb = wts.tile([P, FT], f32)
            nc.sync.dma_start(b1_sb[:], b1_d.ap())
            ones_sb = wts.tile([P, TPB], bf16)
            nc.vector.memset(ones_sb[:], 0.0)
            nc.vector.memset(ones_sb[0:1, :], 1.0)

            a2a_in = [[dram.tile([N_CORES, 64, TPB], bf16, tag=f"ain{b}{h}",
                               name=f"ain{b}{h}") for h in range(HPC)]
                      for b in range(B)]
            a2a_out = [[dram.tile([N_CORES, 64, TPB], bf16, tag=f"aout{b}{h}",
                                name=f"aout{b}{h}") for h in range(HPC)]
                       for b in range(B)]

            def emit_a2a(b, h):
                nc.gpsimd.collective_compute(
                    "AllToAll", ALU.bypass,
                    replica_groups=[list(range(N_CORES))],
                    ins=[a2a_in[b][h].opt()], outs=[a2a_out[b][h].opt()])

            q_sb = [None, None]
            k_sb = [None, None]
            v_sb = [None, None]
            oall_sb = [None, None]
            out1b_sb = [None, None]
            out1T_sb = [None, None]
            h1T_sb = [None, None]

            def gen_qkv(b):
                xt_b = p1.tile([P, KT * T], bf16, tag="xt", bufs=1, name="xt_b")
                for kt in range(KT):
                    nc.sync.dma_start(xt_b[:, kt * T:(kt + 1) * T],
                                      xt_d.ap()[kt, :, b * T:(b + 1) * T])
                yield
                q_sb[b] = p1.tile([P, T], bf16, tag="q", name="q_sb")
                k_sb[b] = p1.tile([P, T], bf16, tag="k", name="k_sb")
                v_sb[b] = p1.tile([P, T], bf16, tag="v", name="v_sb")
                for dst, wofs in ((q_sb[b], 0), (k_sb[b], P)):
                    for half in range(2):
                        pt = psp.tile([P, 1024], f32, tag="mm", name="pt")
                        for kt in range(KT):
                            for c in range(2):
                                ofs = half * 1024 + c * 512
                                nc.tensor.matmul(
                                    pt[:, c * 512:(c + 1) * 512],
                                    wqkv_sb[:, kt * 384 + wofs: kt * 384 + wofs + P],
                                    xt_b[:, kt * T + ofs: kt * T + ofs + 512],
                                    start=(kt == 0), stop=(kt == KT - 1))
                            if kt == 3:
                                yield
                        nc.vector.tensor_copy(
                            dst[:, half * 1024: half * 1024 + 1024], pt[:])
                        yield
                for st in range(ST):
                    pv = psp.tile([P, P], f32, tag="mm", name="pv")
                    for kt in range(KT):
                        nc.tensor.matmul(
                            pv[:],
                            xt_b[:, kt * T + st * P: kt * T + (st + 1) * P],
                            wqkv_sb[:, kt * 384 + 256: kt * 384 + 384],
                            start=(kt == 0), stop=(kt == KT - 1))
                    nc.vector.tensor_copy(v_sb[b][:, st * P:(st + 1) * P], pv[:])
                    yield

            def gen_attn(b):
                for h in range(HPC):
                    hofs = 64 * h
                    o_ps = psp.tile([64, T], f32, tag="o", bufs=1, name="o_ps")
                    for st in range(ST):
                        at_tiles = [None, None]
                        zp = small.tile([P, 2], f32, tag="zp", name="zp")
                        for half in range(2):
                            sc = psp.tile([P, 1024], f32, tag="mm", name="sc")
                            for c in range(2):
                                ofs = half * 1024 + c * 512
                                nc.tensor.matmul(
                                    sc[:, c * 512:(c + 1) * 512],
                                    k_sb[b][hofs:hofs + 64, st * P:(st + 1) * P],
                                    q_sb[b][hofs:hofs + 64, ofs: ofs + 512],
                                    start=True, stop=True)
                            at = p1.tile([P, 1024], bf16, tag="at", bufs=6, name="at")
                            nc.scalar.activation(at[:], sc[:], AF.Exp, scale=0.125,
                                                 accum_out=zp[:, half:half + 1])
                            at_tiles[half] = at
                        yield
                        zs = small.tile([P, 1], f32, tag="zs", name="zs")
                        nc.vector.tensor_add(zs[:], zp[:, 0:1], zp[:, 1:2])
                        zi = small.tile([P, 1], f32, tag="zi", name="zi")
                        nc.vector.reciprocal(zi[:], zs[:])
                        vp = small.tile([P, 64], bf16, tag="vp", name="vp")
                        nc.vector.tensor_scalar_mul(
                            vp[:], v_sb[b][:, st * P + hofs: st * P + hofs + 64], zi[:])
                        for half in range(2):
                            for c in range(2):
                                ofs = half * 1024 + c * 512
                                nc.tensor.matmul(
                                    o_ps[:, ofs: ofs + 512],
                                    vp[:],
                                    at_tiles[half][:, c * 512:(c + 1) * 512],
                                    start=(st == 0), stop=(st == ST - 1))
                        yield
                    ot = p1.tile([64, T], bf16, tag="ot", bufs=2, name="ot")
                    nc.vector.tensor_copy(ot[:], o_ps[:])
                    for j in range(N_CORES):
                        nc.sync.dma_start(a2a_in[b][h][j],
                                          ot[:, j * TPB:(j + 1) * TPB])
                    emit_a2a(b, h)

            i32 = mybir.dt.int32
            magic1 = wts.tile([P, 1], i32)
            nc.vector.memset(magic1[:], 0x5f3759e0)

            def norm_rows(y_ap, ssum, out_ap):
                negmean = small.tile([P, 1], f32, tag="st2", name="negmean")
                nc.scalar.mul(negmean[:], ssum[:], -1.0 / D)
                sq = p2.tile([P, D], bf16, tag="sq", bufs=1, name="sq")
                ssq = small.tile([P, 1], f32, tag="st4", name="ssq")
                nc.scalar.activation(sq[:], y_ap, AF.Square,
                                     bias=negmean[:], accum_out=ssq[:])
                # istd = rsqrt(ssq/(D-1)) via magic-constant seed + 2 Newton
                # steps, all tiny [P,1] DVE ops (keeps ScalarE in one
                # activation table set - no ACT_TABLE_LOAD churn).
                v = small.tile([P, 1], f32, tag="st5", name="v")
                nc.vector.tensor_scalar_mul(v[:], ssq[:], 1.0 / (D - 1))
                yh = small.tile([P, 1], i32, tag="st6", name="yh")
                nc.vector.tensor_scalar(yh[:], v[:].bitcast(i32), 1, None,
                                        op0=ALU.logical_shift_right)
                yn = small.tile([P, 1], i32, tag="st12", name="yn")
                nc.vector.tensor_scalar(yn[:], yh[:], -1, None,
                                        op0=ALU.bitwise_xor)
                y0 = small.tile([P, 1], i32, tag="st7", name="y0")
                nc.vector.tensor_add(y0[:], yn[:], magic1[:])
                istd = y0[:].bitcast(f32)
                for _ in range(2):
                    aa = small.tile([P, 1], f32, tag="st8", name="aa")
                    nc.vector.tensor_mul(aa[:], istd, istd)
                    bb = small.tile([P, 1], f32, tag="st9", name="bb")
                    nc.vector.tensor_mul(bb[:], v[:], aa[:])
                    cc = small.tile([P, 1], f32, tag="st10", name="cc")
                    nc.vector.tensor_scalar(cc[:], bb[:], -0.5, 1.5,
                                            op0=ALU.mult, op1=ALU.add)
                    ny = small.tile([P, 1], f32, tag="st11", name="ny")
                    nc.vector.tensor_mul(ny[:], cc[:], istd)
                    istd = ny[:]
                nc.vector.tensor_scalar(out_ap, y_ap, negmean[:], istd,
                                        op0=ALU.add, op1=ALU.mult)

            def gen_p2a(b):
                # gather attention features for my tokens of batch b
                oall_sb[b] = o1p.tile([P, KT * TPB], bf16, tag="oall", name="oall_sb")
                for kt in range(KT):
                    for h in range(HPC):
                        nc.sync.dma_start(
                            oall_sb[b][64 * h:64 * h + 64, kt * TPB:(kt + 1) * TPB],
                            a2a_out[b][h][kt])
                out1b_sb[b] = o1p.tile([P, TT * D], bf16, tag="out1b", name="out1b_sb")
                out1T_sb[b] = o1p.tile([P, KT * TPB], bf16, tag="out1t", name="out1T_sb")
                for tt in range(TT):
                    pw = psp.tile([P, D], f32, tag="mm", name="pw")
                    for kt in range(KT):
                        for c in range(2):
                            nc.tensor.matmul(
                                pw[:, c * 512:(c + 1) * 512],
                                oall_sb[b][:, kt * TPB + tt * P: kt * TPB + (tt + 1) * P],
                                wo_sb[:, kt * D + c * 512: kt * D + (c + 1) * 512],
                                start=(kt == 0), stop=(kt == KT - 1))
                        if kt % 4 == 3:
                            yield
                    xr = p2.tile([P, D], f32, tag="xr", name="xr")
                    nc.sync.dma_start(xr[:], xres_r[b * TT + tt])
                    y = p2.tile([P, D], f32, tag="y", name="y")
                    ssum = small.tile([P, 1], f32, tag="st1", name="ssum")
                    nc.vector.scalar_tensor_tensor(
                        y[:], pw[:], 0.0, xr[:], op0=ALU.add, op1=ALU.add,
                        accum_out=ssum[:])
                    norm_rows(y[:], ssum, out1b_sb[b][:, tt * D:(tt + 1) * D])
                    yield
                    for kt in range(KT):
                        nc.sync.dma_start_transpose(
                            out1T_sb[b][:, kt * TPB + tt * P: kt * TPB + (tt + 1) * P],
                            out1b_sb[b][:, tt * D + kt * P: tt * D + (kt + 1) * P])
                    yield
                h1T_sb[b] = p2.tile([P, FT * TPB], bf16, tag="h1t", bufs=1, name="h1T_sb")
                for ft in range(FT):
                    w1s = p2.tile([P, KT * P], bf16, tag="w1s", name="w1s")
                    dma3(w1s[:], w1_d.ap()[ft], KT)
                    ph = psp.tile([P, TPB], f32, tag="mm", name="ph")
                    for kt in range(KT):
                        nc.tensor.matmul(
                            ph[:],
                            w1s[:, kt * P:(kt + 1) * P],
                            out1T_sb[b][:, kt * TPB:(kt + 1) * TPB],
                            start=(kt == 0), stop=(kt == KT - 1))
                        if kt % 4 == 3:
                            yield
                    # relu(x + b1) on DVE: (ph + b1) max 0, bf16 out
                    nc.vector.tensor_scalar(
                        h1T_sb[b][:, ft * TPB:(ft + 1) * TPB], ph[:],
                        b1_sb[:, ft:ft + 1], 0.0, op0=ALU.add, op1=ALU.max)

            def gen_ffn(b):
                pfs = [psp.tile([P, D], f32, tag="mm", name=f"pf{tt}")
                       for tt in range(TT)]
                for ft in range(FT + 1):
                    w2s = p2.tile([P, D], bf16, tag="w2s", bufs=8, name="w2s")
                    nc.sync.dma_start(w2s[:], w2_d.ap()[ft])
                    last = ft == FT
                    for tt in range(TT):
                        lhsT = (ones_sb[:, tt * P:(tt + 1) * P] if last else
                                h1T_sb[b][:, ft * TPB + tt * P: ft * TPB + (tt + 1) * P])
                        for c in range(2):
                            nc.tensor.matmul(
                                pfs[tt][:, c * 512:(c + 1) * 512],
                                lhsT,
                                w2s[:, c * 512:(c + 1) * 512],
                                start=(ft == 0), stop=last)
                    if ft % 2 == 1:
                        yield
                for tt in range(TT):
                    y2 = p2.tile([P, D], f32, tag="y", name="y2")
                    ssum = small.tile([P, 1], f32, tag="st1", name="ssum2")
                    nc.vector.scalar_tensor_tensor(
                        y2[:], pfs[tt][:], 0.0, out1b_sb[b][:, tt * D:(tt + 1) * D],
                        op0=ALU.add, op1=ALU.add, accum_out=ssum[:])
                    o2 = p2.tile([P, D], f32, tag="o2", name="o2")
                    norm_rows(y2[:], ssum, o2[:])
                    nc.sync.dma_start(out_r[b * TT + tt], o2[:])
                    yield

            # ---------------- emission schedule ----------------
            drain(gen_qkv(0))
            attn0 = gen_attn(0)
            qkv1 = gen_qkv(1)
            next(qkv1)          # emit xt(b1) load early
            for _ in range(12):  # ~3 s-tiles solo while xt(b1) streams in
                next(attn0)
            zip2(attn0, qkv1, ratio=1)
            attn1 = gen_attn(1)
            for _ in range(16):   # solo prefix: let the A2A-0 halves land
                next(attn1)
            zip2(attn1, gen_p2a(0), ratio=3)
            drain(gen_ffn(0))
            drain(gen_p2a(1))
            drain(gen_ffn(1))

    nc.compile()
    return nc


def _get_nc():
    if "nc" not in _CACHE:
        _CACHE["nc"] = _build()
    return _CACHE["nc"]


def _prep_inputs(x, Wq, Wk, Wv, Wo, W1, b1, W2, b2):
    import ml_dtypes
    bf = ml_dtypes.bfloat16
    x = np.asarray(x, np.float32)
    x2 = np.ascontiguousarray(x.reshape(TOK, D))
    xt = np.ascontiguousarray(x2.T).astype(bf).reshape(KT, P, TOK)
    wo8 = np.ascontiguousarray(np.asarray(Wo, np.float32).astype(bf).reshape(KT, P, D))
    w1t = np.ascontiguousarray(
        np.asarray(W1, np.float32).astype(bf).reshape(KT, P, FT, P).transpose(2, 0, 1, 3))
    b2blk = np.zeros((1, P, D), np.float32)
    b2blk[0, 0, :] = np.asarray(b2, np.float32)
    w2t = np.ascontiguousarray(np.concatenate(
        [np.asarray(W2, np.float32).reshape(FT, P, D), b2blk], axis=0).astype(bf))
    b1t = np.ascontiguousarray(np.asarray(b1, np.float32).reshape(FT, P).T)
    Wq = np.asarray(Wq, np.float32)
    Wk = np.asarray(Wk, np.float32)
    Wv = np.asarray(Wv, np.float32)
    in_maps = []
    for c in range(N_CORES):
        h0 = HPC * c
        wqkv = np.concatenate(
            [Wq[h0], Wq[h0 + 1], Wk[h0], Wk[h0 + 1], Wv[h0], Wv[h0 + 1]],
            axis=1).astype(bf)
        wqkv = np.ascontiguousarray(wqkv.reshape(KT, P, 384))
        xres = np.ascontiguousarray(np.concatenate(
            [x2[c * TPB:(c + 1) * TPB],
             x2[T + c * TPB: T + (c + 1) * TPB]], axis=0))
        in_maps.append({
            "xt": xt, "xres": xres, "wqkv": wqkv, "wo": wo8,
            "w1": w1t, "b1": b1t, "w2": w2t,
        })
    return in_maps


def _assemble(results):
    out = np.empty((TOK, D), np.float32)
    for c in range(N_CORES):
        r = np.asarray(results[c]["out"], np.float32)
        out[c * TPB:(c + 1) * TPB] = r[:TPB]
        out[T + c * TPB: T + (c + 1) * TPB] = r[TPB:]
    return out.reshape(B, T, D)


def kernel(x, Wq, Wk, Wv, Wo, W1, b1, W2, b2):
    from concourse.bass_utils import run_bass_kernel_spmd
    nc = _get_nc()
    in_maps = _prep_inputs(x, Wq, Wk, Wv, Wo, W1, b1, W2, b2)
    res = run_bass_kernel_spmd(nc, in_maps, core_ids=list(range(N_CORES)))
    return _assemble(res.results)


# revision 15
# speedup vs baseline: 1.1469x; 1.0951x over previous
"""Trainium2 Bass kernel for nn_EncoderStack (dense transformer encoder layer).

Strategy (8 NeuronCores, single NEFF launch):
  Attention is head-parallel: each core owns 2 of 16 heads over all 4096
  tokens. scores^T = k q^T per head, softmax over the query axis (free
  dim), denominator folded into v, o^T accumulated in PSUM with both
  heads packed into one [128, T] tile. Per batch, an AllToAll exchanges
  o^T blocks so each core ends up with all 1024 attention features for
  its 256-token slice of that batch. Wo + residual + norm + FFN + norm
  then run token-parallel. b2 is folded into the W2 matmul via a ones
  row; 1/sqrt(var) is computed as exp(-0.5*ln(var)) so every scalar-
  engine op stays in one activation table set (no ~2.7us set switches).

  Emission is software-pipelined: qkv of batch 1 is interleaved into the
  ACT-bound attention of batch 0, and Wo/norm/W1 work for batch 0's
  tokens is interleaved into the attention of batch 1, keeping TensorE
  dense (HAM stays un-throttled).

All matmuls run bf16 with fp32 PSUM accumulation; normalization
statistics stay fp32.
"""

import numpy as np

B, T, D = 2, 2048, 1024
H, DK, DV = 16, 64, 64
FF = 4096
N_CORES = 8
P = 128
TOK = B * T
TPB = T // N_CORES    # 256 tokens per core per batch
HPC = H // N_CORES    # 2 heads per core
KT = D // P           # 8
FT = FF // P          # 32
ST = T // P           # 16
TT = TPB // P         # 2 token-tiles per core per batch

_CACHE = {}


def _build():
    import concourse.bacc as bacc
    import concourse.mybir as mybir
    from concourse import tile

    f32 = mybir.dt.float32
    bf16 = mybir.dt.bfloat16
    AX = mybir.AxisListType
    AF = mybir.ActivationFunctionType
    ALU = mybir.AluOpType

    nc = bacc.Bacc("TRN2", target_bir_lowering=False, debug=False,
                   enable_asserts=True, num_devices=N_CORES)

    xt_d = nc.dram_tensor("xt", [KT, P, TOK], bf16, kind="ExternalInput")
    xres_d = nc.dram_tensor("xres", [2 * TPB, D], f32, kind="ExternalInput")
    wqkv_d = nc.dram_tensor("wqkv", [KT, P, 384], bf16, kind="ExternalInput")
    wo_d = nc.dram_tensor("wo", [KT, P, D], bf16, kind="ExternalInput")
    w1_d = nc.dram_tensor("w1", [FT, KT, P, P], bf16, kind="ExternalInput")
    b1_d = nc.dram_tensor("b1", [P, FT], f32, kind="ExternalInput")
    w2_d = nc.dram_tensor("w2", [FT + 1, P, D], bf16, kind="ExternalInput")
    out_d = nc.dram_tensor("out", [2 * TPB, D], f32, kind="ExternalOutput")

    xres_r = xres_d.ap().rearrange("(a p) d -> a p d", p=P)
    out_r = out_d.ap().rearrange("(a p) d -> a p d", p=P)

    def drain(g):
        for _ in g:
            pass

    def zip2(main, filler, ratio=1):
        while True:
            try:
                next(main)
            except StopIteration:
                drain(filler)
                return
            for _ in range(ratio):
                try:
                    next(filler)
                except StopIteration:
                    drain(main)
                    return

    with tile.TileContext(nc) as tc:
        with tc.tile_pool(name="wts", bufs=1) as wts, \
             tc.tile_pool(name="small", bufs=6) as small, \
             tc.tile_pool(name="o1", bufs=2) as o1p, \
             tc.tile_pool(name="p1", bufs=2) as p1, \
             tc.tile_pool(name="p2", bufs=2) as p2, \
             tc.tile_pool(name="ps", bufs=2, space="PSUM") as psp, \
             tc.tile_pool(name="dram", bufs=1, space="DRAM") as dram:

            def dma3(dst2d, src_ap, a):
                nc.sync.dma_start(
                    dst2d.rearrange("p (a m) -> p a m", a=a),
                    src_ap.rearrange("a p m -> p a m"))

            wqkv_sb = wts.tile([P, KT * 384], bf16)
            dma3(wqkv_sb[:], wqkv_d.ap(), KT)
            wo_sb = wts.tile([P, KT * D], bf16)
            dma3(wo_sb[:], wo_d.ap(), KT)
            b1_sb = wts.tile([P, FT], f32)
            nc.sync.dma_start(b1_sb[:], b1_d.ap())
            ones_sb = wts.tile([P, TPB], bf16)
            nc.vector.memset(ones_sb[:], 0.0)
            nc.vector.memset(ones_sb[0:1, :], 1.0)

            a2a_in = [[dram.tile([N_CORES, 64, TPB], bf16, tag=f"ain{b}{h}",
                               name=f"ain{b}{h}") for h in range(HPC)]
                      for b in range(B)]
            a2a_out = [[dram.tile([N_CORES, 64, TPB], bf16, tag=f"aout{b}{h}",
                                name=f"aout{b}{h}") for h in range(HPC)]
                       for b in range(B)]

            def emit_a2a(b, h):
                nc.gpsimd.collective_compute(
                    "AllToAll", ALU.bypass,
                    replica_groups=[list(range(N_CORES))],
                    ins=[a2a_in[b][h].opt()], outs=[a2a_out[b][h].opt()])

            out1T_all = o1p.tile([P, KT * 2 * TPB], bf16, tag="out1t",
                                 bufs=1, name="out1T_all")
            h1T_all = p2.tile([P, FT * 2 * TPB], bf16, tag="h1t", bufs=1,
                              name="h1T_all")
            q_sb = [None, None]
            k_sb = [None, None]
            v_sb = [None, None]
            oall_sb = [None, None]
            out1b_sb = [None, None]
            out1T_sb = [None, None]
            h1T_sb = [None, None]

            def gen_qkv(b):
                xt_b = p1.tile([P, KT * T], bf16, tag="xt", bufs=1, name="xt_b")
                for kt in range(KT):
                    nc.sync.dma_start(xt_b[:, kt * T:(kt + 1) * T],
                                      xt_d.ap()[kt, :, b * T:(b + 1) * T])
                yield
                q_sb[b] = p1.tile([P, T], bf16, tag="q", name="q_sb")
                k_sb[b] = p1.tile([P, T], bf16, tag="k", name="k_sb")
                v_sb[b] = p1.tile([P, T], bf16, tag="v", name="v_sb")
                for dst, wofs in ((q_sb[b], 0), (k_sb[b], P)):
                    for half in range(2):
                        pt = psp.tile([P, 1024], f32, tag="mm", name="pt")
                        for kt in range(KT):
                            for c in range(2):
                                ofs = half * 1024 + c * 512
                                nc.tensor.matmul(
                                    pt[:, c * 512:(c + 1) * 512],
                                    wqkv_sb[:, kt * 384 + wofs: kt * 384 + wofs + P],
                                    xt_b[:, kt * T + ofs: kt * T + ofs + 512],
                                    start=(kt == 0), stop=(kt == KT - 1))
                            if kt == 3:
                                yield
                        nc.vector.tensor_copy(
                            dst[:, half * 1024: half * 1024 + 1024], pt[:])
                        yield
                for st in range(ST):
                    pv = psp.tile([P, P], f32, tag="mm", name="pv")
                    for kt in range(KT):
                        nc.tensor.matmul(
                            pv[:],
                            xt_b[:, kt * T + st * P: kt * T + (st + 1) * P],
                            wqkv_sb[:, kt * 384 + 256: kt * 384 + 384],
                            start=(kt == 0), stop=(kt == KT - 1))
                    nc.vector.tensor_copy(v_sb[b][:, st * P:(st + 1) * P], pv[:])
                    yield

            def gen_attn(b):
                for h in range(HPC):
                    hofs = 64 * h
                    o_ps = psp.tile([64, T], f32, tag="o", bufs=1, name="o_ps")
                    for st in range(ST):
                        at_tiles = [None, None]
                        zp = small.tile([P, 2], f32, tag="zp", name="zp")
                        for half in range(2):
                            sc = psp.tile([P, 1024], f32, tag="mm", name="sc")
                            for c in range(2):
                                ofs = half * 1024 + c * 512
                                nc.tensor.matmul(
                                    sc[:, c * 512:(c + 1) * 512],
                                    k_sb[b][hofs:hofs + 64, st * P:(st + 1) * P],
                                    q_sb[b][hofs:hofs + 64, ofs: ofs + 512],
                                    start=True, stop=True)
                            at = p1.tile([P, 1024], bf16, tag="at", bufs=6, name="at")
                            nc.scalar.activation(at[:], sc[:], AF.Exp, scale=0.125,
                                                 accum_out=zp[:, half:half + 1])
                            at_tiles[half] = at
                        yield
                        zs = small.tile([P, 1], f32, tag="zs", name="zs")
                        nc.vector.tensor_add(zs[:], zp[:, 0:1], zp[:, 1:2])
                        zi = small.tile([P, 1], f32, tag="zi", name="zi")
                        nc.vector.reciprocal(zi[:], zs[:])
                        vp = small.tile([P, 64], bf16, tag="vp", name="vp")
                        nc.vector.tensor_scalar_mul(
                            vp[:], v_sb[b][:, st * P + hofs: st * P + hofs + 64], zi[:])
                        for half in range(2):
                            for c in range(2):
                                ofs = half * 1024 + c * 512
                                nc.tensor.matmul(
                                    o_ps[:, ofs: ofs + 512],
                                    vp[:],
                                    at_tiles[half][:, c * 512:(c + 1) * 512],
                                    start=(st == 0), stop=(st == ST - 1))
                        yield
                    ot = p1.tile([64, T], bf16, tag="ot", bufs=2, name="ot")
                    nc.vector.tensor_copy(ot[:], o_ps[:])
                    for j in range(N_CORES):
                        nc.sync.dma_start(a2a_in[b][h][j],
                                          ot[:, j * TPB:(j + 1) * TPB])
                    emit_a2a(b, h)

            i32 = mybir.dt.int32
            magic1 = wts.tile([P, 1], i32)
            nc.vector.memset(magic1[:], 0x5f3759e0)

            def norm_rows(y_ap, ssum, out_ap):
                negmean = small.tile([P, 1], f32, tag="st2", name="negmean")
                nc.scalar.mul(negmean[:], ssum[:], -1.0 / D)
                sq = p2.tile([P, D], bf16, tag="sq", bufs=1, name="sq")
                ssq = small.tile([P, 1], f32, tag="st4", name="ssq")
                nc.scalar.activation(sq[:], y_ap, AF.Square,
                                     bias=negmean[:], accum_out=ssq[:])
                # istd = rsqrt(ssq/(D-1)) via magic-constant seed + 2 Newton
                # steps, all tiny [P,1] DVE ops (keeps ScalarE in one
                # activation table set - no ACT_TABLE_LOAD churn).
                v = small.tile([P, 1], f32, tag="st5", name="v")
                nc.vector.tensor_scalar_mul(v[:], ssq[:], 1.0 / (D - 1))
                yh = small.tile([P, 1], i32, tag="st6", name="yh")
                nc.vector.tensor_scalar(yh[:], v[:].bitcast(i32), 1, None,
                                        op0=ALU.logical_shift_right)
                yn = small.tile([P, 1], i32, tag="st12", name="yn")
                nc.vector.tensor_scalar(yn[:], yh[:], -1, None,
                                        op0=ALU.bitwise_xor)
                y0 = small.tile([P, 1], i32, tag="st7", name="y0")
                nc.vector.tensor_add(y0[:], yn[:], magic1[:])
                istd = y0[:].bitcast(f32)
                for _ in range(2):
                    aa = small.tile([P, 1], f32, tag="st8", name="aa")
                    nc.vector.tensor_mul(aa[:], istd, istd)
                    bb = small.tile([P, 1], f32, tag="st9", name="bb")
                    nc.vector.tensor_mul(bb[:], v[:], aa[:])
                    cc = small.tile([P, 1], f32, tag="st10", name="cc")
                    nc.vector.tensor_scalar(cc[:], bb[:], -0.5, 1.5,
                                            op0=ALU.mult, op1=ALU.add)
                    ny = small.tile([P, 1], f32, tag="st11", name="ny")
                    nc.vector.tensor_mul(ny[:], cc[:], istd)
                    istd = ny[:]
                nc.vector.tensor_scalar(out_ap, y_ap, negmean[:], istd,
                                        op0=ALU.add, op1=ALU.mult)

            def gen_p2a(b):
                # gather attention features for my tokens of batch b
                oall_sb[b] = o1p.tile([P, KT * TPB], bf16, tag="oall", name="oall_sb")
                for kt in range(KT):
                    for h in range(HPC):
                        nc.sync.dma_start(
                            oall_sb[b][64 * h:64 * h + 64, kt * TPB:(kt + 1) * TPB],
                            a2a_out[b][h][kt])
                out1b_sb[b] = o1p.tile([P, TT * D], bf16, tag="out1b", name="out1b_sb")
                for tt in range(TT):
                    pw = psp.tile([P, D], f32, tag="mm", name="pw")
                    for kt in range(KT):
                        for c in range(2):
                            nc.tensor.matmul(
                                pw[:, c * 512:(c + 1) * 512],
                                oall_sb[b][:, kt * TPB + tt * P: kt * TPB + (tt + 1) * P],
                                wo_sb[:, kt * D + c * 512: kt * D + (c + 1) * 512],
                                start=(kt == 0), stop=(kt == KT - 1))
                        if kt % 4 == 3:
                            yield
                    xr = p2.tile([P, D], f32, tag="xr", name="xr")
                    nc.sync.dma_start(xr[:], xres_r[b * TT + tt])
                    y = p2.tile([P, D], f32, tag="y", name="y")
                    ssum = small.tile([P, 1], f32, tag="st1", name="ssum")
                    nc.vector.scalar_tensor_tensor(
                        y[:], pw[:], 0.0, xr[:], op0=ALU.add, op1=ALU.add,
                        accum_out=ssum[:])
                    norm_rows(y[:], ssum, out1b_sb[b][:, tt * D:(tt + 1) * D])
                    yield
                    for kt in range(KT):
                        nc.sync.dma_start_transpose(
                            out1T_all[:, kt * 2 * TPB + b * TPB + tt * P:
                                      kt * 2 * TPB + b * TPB + (tt + 1) * P],
                            out1b_sb[b][:, tt * D + kt * P: tt * D + (kt + 1) * P])
                    yield

            def gen_h1():
                for ft in range(FT):
                    w1s = p2.tile([P, KT * P], bf16, tag="w1s", name="w1s")
                    dma3(w1s[:], w1_d.ap()[ft], KT)
                    ph = psp.tile([P, 2 * TPB], f32, tag="o", bufs=1, name="ph")
                    for kt in range(KT):
                        nc.tensor.matmul(
                            ph[:],
                            w1s[:, kt * P:(kt + 1) * P],
                            out1T_all[:, kt * 2 * TPB:(kt + 1) * 2 * TPB],
                            start=(kt == 0), stop=(kt == KT - 1))
                    nc.vector.tensor_scalar(
                        h1T_all[:, ft * 2 * TPB:(ft + 1) * 2 * TPB], ph[:],
                        b1_sb[:, ft:ft + 1], 0.0, op0=ALU.add, op1=ALU.max)
                    yield

            def gen_ffn(b):
                pfs = [psp.tile([P, D], f32, tag="mm", name=f"pf{tt}")
                       for tt in range(TT)]
                for ft in range(FT + 1):
                    w2s = p2.tile([P, D], bf16, tag="w2s", bufs=8, name="w2s")
                    nc.sync.dma_start(w2s[:], w2_d.ap()[ft])
                    last = ft == FT
                    for tt in range(TT):
                        lhsT = (ones_sb[:, tt * P:(tt + 1) * P] if last else
                                h1T_all[:, ft * 2 * TPB + b * TPB + tt * P:
                                        ft * 2 * TPB + b * TPB + (tt + 1) * P])
                        for c in range(2):
                            nc.tensor.matmul(
                                pfs[tt][:, c * 512:(c + 1) * 512],
                                lhsT,
                                w2s[:, c * 512:(c + 1) * 512],
                                start=(ft == 0), stop=last)
                    yield
                for tt in range(TT):
                    y2 = p2.tile([P, D], f32, tag="y", name="y2")
                    ssum = small.tile([P, 1], f32, tag="st1", name="ssum2")
                    nc.vector.scalar_tensor_tensor(
                        y2[:], pfs[tt][:], 0.0, out1b_sb[b][:, tt * D:(tt + 1) * D],
                        op0=ALU.add, op1=ALU.add, accum_out=ssum[:])
                    o2 = p2.tile([P, D], f32, tag="o2", name="o2")
                    norm_rows(y2[:], ssum, o2[:])
                    nc.sync.dma_start(out_r[b * TT + tt], o2[:])
                    yield

            # ---------------- emission schedule ----------------
            drain(gen_qkv(0))
            attn0 = gen_attn(0)
            qkv1 = gen_qkv(1)
            next(qkv1)          # emit xt(b1) load early
            for _ in range(12):  # ~3 s-tiles solo while xt(b1) streams in
                next(attn0)
            zip2(attn0, qkv1, ratio=1)
            attn1 = gen_attn(1)
            for _ in range(16):   # solo prefix: let the A2A-0 halves land
                next(attn1)
            zip2(attn1, gen_p2a(0), ratio=2)
            drain(gen_p2a(1))
            h1g = gen_h1()
            next(h1g)
            next(h1g)
            zip2(h1g, gen_ffn(0), ratio=1)
            drain(gen_ffn(1))

    nc.compile()
    return nc


def _get_nc():
    if "nc" not in _CACHE:
        _CACHE["nc"] = _build()
    return _CACHE["nc"]


def _prep_inputs(x, Wq, Wk, Wv, Wo, W1, b1, W2, b2):
    import ml_dtypes
    bf = ml_dtypes.bfloat16
    x = np.asarray(x, np.float32)
    x2 = np.ascontiguousarray(x.reshape(TOK, D))
    xt = np.ascontiguousarray(x2.T).astype(bf).reshape(KT, P, TOK)
    wo8 = np.ascontiguousarray(np.asarray(Wo, np.float32).astype(bf).reshape(KT, P, D))
    w1t = np.ascontiguousarray(
        np.asarray(W1, np.float32).astype(bf).reshape(KT, P, FT, P).transpose(2, 0, 1, 3))
    b2blk = np.zeros((1, P, D), np.float32)
    b2blk[0, 0, :] = np.asarray(b2, np.float32)
    w2t = np.ascontiguousarray(np.concatenate(
        [np.asarray(W2, np.float32).reshape(FT, P, D), b2blk], axis=0).astype(bf))
    b1t = np.ascontiguousarray(np.asarray(b1, np.float32).reshape(FT, P).T)
    Wq = np.asarray(Wq, np.float32)
    Wk = np.asarray(Wk, np.float32)
    Wv = np.asarray(Wv, np.float32)
    in_maps = []
    for c in range(N_CORES):
        h0 = HPC * c
        wqkv = np.concatenate(
            [Wq[h0], Wq[h0 + 1], Wk[h0], Wk[h0 + 1], Wv[h0], Wv[h0 + 1]],
            axis=1).astype(bf)
        wqkv = np.ascontiguousarray(wqkv.reshape(KT, P, 384))
        xres = np.ascontiguousarray(np.concatenate(
            [x2[c * TPB:(c + 1) * TPB],
             x2[T + c * TPB: T + (c + 1) * TPB]], axis=0))
        in_maps.append({
            "xt": xt, "xres": xres, "wqkv": wqkv, "wo": wo8,
            "w1": w1t, "b1": b1t, "w2": w2t,
        })
    return in_maps


def _assemble(results):
    out = np.empty((TOK, D), np.float32)
    for c in range(N_CORES):
        r = np.asarray(results[c]["out"], np.float32)
        out[c * TPB:(c + 1) * TPB] = r[:TPB]
        out[T + c * TPB: T + (c + 1) * TPB] = r[TPB:]
    return out.reshape(B, T, D)


def kernel(x, Wq, Wk, Wv, Wo, W1, b1, W2, b2):
    from concourse.bass_utils import run_bass_kernel_spmd
    nc = _get_nc()
    in_maps = _prep_inputs(x, Wq, Wk, Wv, Wo, W1, b1, W2, b2)
    res = run_bass_kernel_spmd(nc, in_maps, core_ids=list(range(N_CORES)))
    return _assemble(res.results)
